# revision 1
# baseline (speedup 1.0000x reference)
"""Trainium2 Bass kernel for GammaLambdaLearner lambda-return scan.

Computes, per batch row b (backward over time t = S-1 .. 0):

    gamma   = max(tanh(raw_gamma), 1e-8)            # scalar
    lambd_t = max(tanh(raw_lambd[t]), 1e-8)         # [S]
    ret[t]  = r[t] + gamma*(1-d[t])*((1-lambd_t)*v[t+1] + lambd_t*ret[t+1])
    ret[S]  := v[S]   (bootstrap carry)

which is the first-order linear recurrence ret[t] = b[t] + a[t]*ret[t+1] with

    a[t] = gamma*lambd_t*(1-d[t])
    b[t] = r[t] + gamma*(1-lambd_t)*(1-d[t])*v[t+1]

Mapping: batch is data-parallel across the 8 NeuronCores (1024 rows/core),
and within a core across the 128 SBUF partitions (8 row-tiles of
[128, 2048]).  Time lives in the free dimension; the recurrence runs on the
DVE TensorTensorScan instruction with *reversed* access patterns on all
three operands, so the backward-in-time order and the output reversal are
both free.

Engine balance per tile (f32 ⇒ all DVE ops at 1 elem/lane/cycle):
  ACT    u = 1 - d                       (affine copy, immediates)
  GPSIMD a = u * (gamma*lambd)           (plain TT mult — HW-supported)
  DVE    w = (d == 0) * v_next           (fused scalar_tensor_tensor)
  DVE    w *= gamma*(1-lambd)
  DVE    w += r
  DVE    scan (reversed APs, initial = v[S])
Loads go out on the SP HWDGE ring, stores on the ACT HWDGE ring, so the two
hardware descriptor-generation rings run concurrently.  d==0.0/1.0 exactly
(the oracle casts a boolean), so (1-d) == is_equal(d, 0).
"""

import numpy as np

import concourse.bass as bass
import concourse.tile as tile
import concourse.mybir as mybir
from concourse import bacc
from concourse.bass_utils import run_bass_kernel_spmd

B, S = 8192, 2048
N_CORES = 8
R = B // N_CORES          # rows per core
P = 128                   # SBUF partitions
NT = R // P               # row-tiles per core
EPS = 1e-8

F32 = mybir.dt.float32
ALU = mybir.AluOpType


def build_kernel(rows=R, s=S, bufs=3, a_path="act_gpsimd", split_last=4):
    """a_path: "dve" (5 DVE ops) or "act_gpsimd" (4 DVE ops; u on ACT,
    a-multiply on GPSIMD).  split_last: split the final row-tile's compute /
    scan / store into time-chunks (chained scan carries) so the tail store
    overlaps the tail compute instead of idling the DMA engines.  Chunks
    taper (1024/512/256/256) — the last dependency chain is the shortest.
    """
    nt = rows // P
    nc = bacc.Bacc(
        "TRN2",
        target_bir_lowering=False,
        debug=False,
        enable_asserts=False,
        num_devices=N_CORES,
    )
    values = nc.dram_tensor("values", [rows, s + 1], F32, kind="ExternalInput").ap()
    rewards = nc.dram_tensor("rewards", [rows, s], F32, kind="ExternalInput").ap()
    dones = nc.dram_tensor("dones", [rows, s], F32, kind="ExternalInput").ap()
    raw_gamma = nc.dram_tensor("raw_gamma", [1, 1], F32, kind="ExternalInput").ap()
    raw_lambd = nc.dram_tensor("raw_lambd", [1, s], F32, kind="ExternalInput").ap()
    ret = nc.dram_tensor("ret", [rows, s], F32, kind="ExternalOutput").ap()

    with tile.TileContext(nc) as tc:
        with (
            tc.tile_pool(name="const", bufs=1) as const_pool,
            tc.tile_pool(name="ins", bufs=bufs) as in_pool,
            tc.tile_pool(name="tmp", bufs=bufs) as tmp_pool,
            tc.tile_pool(name="out", bufs=bufs) as out_pool,
        ):
            # ---- one-time parameter prep (tiny [1, s] rows) ----
            # prep loads ride the ACT HWDGE ring so the tiny transfers don't
            # delay the first 1 MiB load on the SP ring (FIFO per ring).
            lam = const_pool.tile([1, s], F32, tag="lam")
            nc.scalar.dma_start(lam[:], raw_lambd[:])
            g = const_pool.tile([1, 1], F32, tag="g")
            nc.scalar.dma_start(g[:], raw_gamma[:])

            nc.scalar.activation(lam[:], lam[:], mybir.ActivationFunctionType.Tanh)
            nc.scalar.activation(g[:], g[:], mybir.ActivationFunctionType.Tanh)
            nc.vector.tensor_scalar_max(g[:], g[:], EPS)

            # glam = max(tanh(raw_lambd), eps) * gamma      (fused clamp+scale)
            glam = const_pool.tile([1, s], F32, tag="glam")
            nc.vector.tensor_scalar(
                glam[:], lam[:], EPS, g[:, 0:1], op0=ALU.max, op1=ALU.mult
            )
            # gmlam = gamma - glam = gamma*(1-lambda)
            gmlam = const_pool.tile([1, s], F32, tag="gmlam")
            nc.vector.tensor_scalar(
                gmlam[:], glam[:], -1.0, g[:, 0:1], op0=ALU.mult, op1=ALU.add
            )

            glamR = const_pool.tile([P, s], F32, tag="glamR")
            nc.gpsimd.partition_broadcast(glamR[:], glam[:])
            gmlamR = const_pool.tile([P, s], F32, tag="gmlamR")
            nc.gpsimd.partition_broadcast(gmlamR[:], gmlam[:])

            # ---- main loop over row-tiles ----
            for i in range(nt):
                rs = slice(i * P, (i + 1) * P)
                d = in_pool.tile([P, s], F32, tag="d")
                nc.sync.dma_start(d[:], dones[rs, :])
                # full 2049-column rows: one perfectly contiguous block per
                # 128-row slab (vs. strided rows for the [1:s+1] slice)
                vf = in_pool.tile([P, s + 1], F32, tag="vf")
                nc.sync.dma_start(vf[:], values[rs, :])
                vn = vf[:, 1 : s + 1]   # v[t+1] view
                r = in_pool.tile([P, s], F32, tag="r")
                nc.sync.dma_start(r[:], rewards[rs, :])

                a = tmp_pool.tile([P, s], F32, tag="a")
                u = tmp_pool.tile([P, s], F32, tag="u")
                o = out_pool.tile([P, s], F32, tag="o")

                # final tile: pipeline the tail in time-chunks (high chunk
                # first — the backward scan's carry flows high -> low),
                # tapering so the final chain is shortest
                if split_last and i == nt - 1 and s % 8 == 0:
                    bounds = [0, s // 8, s // 4, s // 2, s]
                else:
                    bounds = [0, s]
                for pc in range(len(bounds) - 2, -1, -1):
                    cs = slice(bounds[pc], bounds[pc + 1])
                    # a = (1 - d) * gamma*lambda   (off the DVE where possible)
                    if a_path == "act_gpsimd":
                        nc.scalar.activation(
                            u[:, cs],
                            d[:, cs],
                            mybir.ActivationFunctionType.Copy,
                            bias=1.0,
                            scale=-1.0,
                        )
                        nc.gpsimd.tensor_mul(a[:, cs], u[:, cs], glamR[:, cs])
                    else:
                        nc.vector.scalar_tensor_tensor(
                            a[:, cs], d[:, cs], 0.0, glamR[:, cs],
                            op0=ALU.is_equal, op1=ALU.mult,
                        )

                    # d := (d == 0) * v_next     (in-place; last read of raw d)
                    nc.vector.scalar_tensor_tensor(
                        d[:, cs], d[:, cs], 0.0, vn[:, cs],
                        op0=ALU.is_equal, op1=ALU.mult,
                    )
                    # d := d * gamma*(1-lambda)
                    nc.vector.tensor_mul(d[:, cs], d[:, cs], gmlamR[:, cs])
                    # d := d + r                  (= b)
                    nc.vector.tensor_add(d[:, cs], d[:, cs], r[:, cs])

                    # backward scan via reversed access patterns: iteration k
                    # reads a/b at time hi-1-k and writes out there too, so
                    # state = a[t]*state + b[t] walks t = hi-1 .. lo.  The
                    # carry enters from v[S] (top chunk) or the previous
                    # chunk's first output column.
                    hi = bounds[pc + 1]
                    if hi == s:
                        init = vn[:, s - 1 : s]
                    else:
                        init = o[:, hi : hi + 1]
                    nc.vector.tensor_tensor_scan(
                        o[:, cs][:, ::-1],
                        a[:, cs][:, ::-1],
                        d[:, cs][:, ::-1],
                        init,
                        op0=ALU.mult,
                        op1=ALU.add,
                    )
                    # stores ride the ACT HWDGE ring, loads the SP ring
                    nc.scalar.dma_start(ret[rs, cs], o[:, cs])

    nc.compile()
    return nc


_nc_cache = {}


def _get_nc():
    if "nc" not in _nc_cache:
        _nc_cache["nc"] = build_kernel()
    return _nc_cache["nc"]


def kernel(values, rewards, dones, raw_gamma, raw_lambd, trace=False):
    values = np.ascontiguousarray(values, np.float32).reshape(B, S + 1)
    rewards = np.ascontiguousarray(rewards, np.float32).reshape(B, S)
    dones = np.ascontiguousarray(dones, np.float32).reshape(B, S)
    g = np.ascontiguousarray(raw_gamma, np.float32).reshape(1, 1)
    lam = np.ascontiguousarray(raw_lambd, np.float32).reshape(1, S)

    in_maps = []
    for c in range(N_CORES):
        rs = slice(c * R, (c + 1) * R)
        in_maps.append(
            {
                "values": values[rs],
                "rewards": rewards[rs],
                "dones": dones[rs],
                "raw_gamma": g,
                "raw_lambd": lam,
            }
        )

    nc = _get_nc()
    if not trace:
        # NTFF profiling needs axon hooks that may be absent; force it off
        # unless explicitly requested
        import os

        os.environ["BASS_NEVER_TRACE"] = "1"
    try:
        res = run_bass_kernel_spmd(
            nc, in_maps, core_ids=list(range(N_CORES)), trace=trace
        )
    except Exception:
        # transient NRT/axon hiccups (e.g. a wedged exec unit from a prior
        # run) are recoverable on retry
        res = run_bass_kernel_spmd(
            nc, in_maps, core_ids=list(range(N_CORES)), trace=trace
        )
    out = np.concatenate([res.results[c]["ret"] for c in range(N_CORES)], axis=0)
    if trace:
        kernel.last_results = res
    return out.reshape(B, S, 1)



# revision 22
# speedup vs baseline: 1.9212x; 1.9212x over previous
"""Trainium2 Bass kernel for GammaLambdaLearner lambda-return scan.

Computes, per batch row b (backward over time t = S-1 .. 0):

    gamma   = max(tanh(raw_gamma), 1e-8)            # scalar
    lambd_t = max(tanh(raw_lambd[t]), 1e-8)         # [S]
    ret[t]  = r[t] + gamma*(1-d[t])*((1-lambd_t)*v[t+1] + lambd_t*ret[t+1])
    ret[S]  := v[S]   (bootstrap carry)

which is the first-order linear recurrence ret[t] = b[t] + a[t]*ret[t+1] with

    a[t] = gamma*lambd_t*(1-d[t])
    b[t] = r[t] + gamma*(1-lambd_t)*(1-d[t])*v[t+1]

Mapping: batch is data-parallel across the 8 NeuronCores (1024 rows/core),
and within a core across the 128 SBUF partitions (8 row-tiles of
[128, 2048]).  Time lives in the free dimension; the recurrence runs on
DVE TensorTensorScan with *reversed* access patterns, so the backward-in-
time order and the output reversal are both free.

The kernel is DMA-bound, so inputs/outputs ride in reduced precision
(tolerance is 2e-2; fp16 keeps the result at ~1e-3):
  v_next  fp16  [R, S]   (values[:, 1:]; col 0 of values is never used)
  rewards fp16  [R, S]
  dones   fp8e4 [R, S]   (exact: dones are 0.0/1.0)
  ret     fp16  [R, S]
Per-core traffic drops 33.6 MB -> 14.7 MB (~41 us at the 360 GB/s DMA
roofline vs ~94 us for f32).

Two compiled variants, dispatched on the host by inspecting raw_lambd:

* uniform lambda (the GammaLambdaLearner init: raw_lambd = atanh(0.9)*ones):
  gamma*lambd and gamma*(1-lambd) are per-partition scalars, so per tile
    ACT     u = 1 - d ;  a = glam * u          (two affine ops)
    GPSIMD  q = u * v_next                      (TT mult)
    DVE     w = q * gmlam (tensor_scalar, 4x) ; b = w + r ; scan
  and every engine stays under the DMA roofline.

* general lambda: glam/gmlam are [S] rows broadcast to [128, S] once;
  the column-varying multiplies must be tensor-tensor ops
    ACT     u = 1 - d
    GPSIMD  a = u * glamR                       (TT mult)
    DVE     q = u*vn ; w = q*gmlamR ; b = w+r ; scan
  (mildly DVE-bound: ~46 us.)
"""

import numpy as np
import ml_dtypes

import concourse.bass as bass
import concourse.tile as tile
import concourse.mybir as mybir
from concourse import bacc
from concourse.bass_utils import run_bass_kernel_spmd

B, S = 8192, 2048
N_CORES = 8
R = B // N_CORES          # rows per core
P = 128                   # SBUF partitions
NT = R // P               # row-tiles per core
EPS = 1e-8

F32 = mybir.dt.float32
F16 = mybir.dt.float16
F8 = mybir.dt.float8e4
ALU = mybir.AluOpType
ACTF = mybir.ActivationFunctionType
FP8_NP = ml_dtypes.float8_e4m3fn


def build_kernel(rows=R, s=S, bufs=6, tmp_bufs=5, out_bufs=4, uniform=True,
                 split_last=4):
    nt = rows // P
    nc = bacc.Bacc(
        "TRN2",
        target_bir_lowering=False,
        debug=False,
        enable_asserts=False,
        num_devices=N_CORES,
    )
    s_lam = 1 if uniform else s
    vnext = nc.dram_tensor("vnext", [rows, s], F16, kind="ExternalInput").ap()
    rewards = nc.dram_tensor("rewards", [rows, s], F16, kind="ExternalInput").ap()
    dones = nc.dram_tensor("dones", [rows, s], F8, kind="ExternalInput").ap()
    raw_gamma = nc.dram_tensor("raw_gamma", [1, 1], F32, kind="ExternalInput").ap()
    raw_lambd = nc.dram_tensor("raw_lambd", [1, s_lam], F32, kind="ExternalInput").ap()
    ret = nc.dram_tensor("ret", [rows, s], F16, kind="ExternalOutput").ap()

    with tile.TileContext(nc) as tc:
        with (
            tc.tile_pool(name="const", bufs=1) as const_pool,
            tc.tile_pool(name="ins", bufs=bufs) as in_pool,
            tc.tile_pool(name="tmp", bufs=tmp_bufs) as tmp_pool,
            tc.tile_pool(name="out", bufs=out_bufs) as out_pool,
        ):
            # ---- one-time parameter prep ----
            # prep loads ride the ACT HWDGE ring so the tiny transfers don't
            # delay the first big load on the SP ring (FIFO per ring).
            lam = const_pool.tile([1, s_lam], F32, tag="lam")
            nc.scalar.dma_start(lam[:], raw_lambd[:])
            g = const_pool.tile([1, 1], F32, tag="g")
            nc.scalar.dma_start(g[:], raw_gamma[:])

            nc.scalar.activation(g[:], g[:], ACTF.Tanh)
            nc.scalar.activation(lam[:], lam[:], ACTF.Tanh)
            nc.vector.tensor_scalar_max(g[:], g[:], EPS)

            # glam = max(tanh(raw_lambd), eps) * gamma      (fused clamp+scale)
            glam = const_pool.tile([1, s_lam], F32, tag="glam")
            nc.vector.tensor_scalar(
                glam[:], lam[:], EPS, g[:, 0:1], op0=ALU.max, op1=ALU.mult
            )
            # gmlam = gamma - glam = gamma*(1-lambda)
            gmlam = const_pool.tile([1, s_lam], F32, tag="gmlam")
            nc.vector.tensor_scalar(
                gmlam[:], glam[:], -1.0, g[:, 0:1], op0=ALU.mult, op1=ALU.add
            )

            if uniform:
                # per-partition scalar columns [P, 1]
                glamC = const_pool.tile([P, 1], F32, tag="glamC")
                nc.gpsimd.partition_broadcast(glamC[:], glam[:])
                gmlamC = const_pool.tile([P, 1], F32, tag="gmlamC")
                nc.gpsimd.partition_broadcast(gmlamC[:], gmlam[:])
            else:
                glamR = const_pool.tile([P, s], F16, tag="glamR")
                glam16 = const_pool.tile([1, s], F16, tag="glam16")
                nc.scalar.copy(glam16[:], glam[:])
                nc.gpsimd.partition_broadcast(glamR[:], glam16[:])
                gmlamR = const_pool.tile([P, s], F16, tag="gmlamR")
                gmlam16 = const_pool.tile([1, s], F16, tag="gmlam16")
                nc.scalar.copy(gmlam16[:], gmlam[:])
                nc.gpsimd.partition_broadcast(gmlamR[:], gmlam16[:])

            # ---- main loop over row-tiles ----
            # Chunking: the backward scan's carry flows high -> low, so time
            # chunks are processed high-first, with the HIGH chunks smallest
            # (the first chunk gates the whole carry chain).  Loads stay
            # full-width (short chunked transfers would be HWDGE-bound).
            # For the tail tiles the elementwise stages move off DVE entirely
            # (uniform variant: u/a/w on ACT via per-partition scale, q/b on
            # GPSIMD) so the drain is just the serial scan chain.
            if split_last and s % 8 == 0:
                # small chunks at BOTH ends of the last tile: the high end
                # gates the carry chain, the low end is the final store
                c = s // 8
                tail_bounds = [0, c, 2 * c, 6 * c, 7 * c, s]
                nsplits = {0: [0, s // 2, s], nt - 2: [0, s // 2, s],
                           nt - 1: tail_bounds}
            else:
                nsplits = {}
            # All loads are emitted first on the SP ring: the in_pool depth
            # gives a bufs-deep prefetch window, and stores (emitted later,
            # also on SP) can never head-of-line-block a load or an ACT op
            # while waiting for their scan.
            ins = []
            for i in range(nt):
                rs = slice(i * P, (i + 1) * P)
                d = in_pool.tile([P, s], F8, tag="d")
                nc.sync.dma_start(d[:], dones[rs, :])
                vn = in_pool.tile([P, s], F16, tag="vn")
                nc.sync.dma_start(vn[:], vnext[rs, :])
                r = in_pool.tile([P, s], F16, tag="r")
                nc.sync.dma_start(r[:], rewards[rs, :])
                ins.append((d, vn, r))

            pending_stores = []
            for i in range(nt):
                rs = slice(i * P, (i + 1) * P)
                bounds = nsplits.get(i, [0, s])
                nchunks = len(bounds) - 1
                tail = uniform and split_last and i >= nt - 2
                d, vn, r = ins[i]

                u = tmp_pool.tile([P, s], F16, tag="u")
                a = tmp_pool.tile([P, s], F16, tag="a")
                q = tmp_pool.tile([P, s], F16, tag="q")
                o = out_pool.tile([P, s], F16, tag="o")

                # flush stores three tiles behind: by then their scans are
                # done, so they never stall the ACT SEQ ahead of tail u/a ops
                while pending_stores and pending_stores[0][0] <= i - 3:
                    _, dst, src = pending_stores.pop(0)
                    nc.scalar.dma_start(dst, src)

                for pc in range(nchunks - 1, -1, -1):
                    cs = slice(bounds[pc], bounds[pc + 1])
                    # u = 1 - d  (fp8 -> fp16 conversion folded in)
                    nc.scalar.activation(
                        u[:, cs], d[:, cs], ACTF.Copy, bias=1.0, scale=-1.0
                    )
                    # a = u * gamma*lambda
                    if tail:
                        nc.scalar.activation(
                            a[:, cs], u[:, cs], ACTF.Copy, scale=glamC[:]
                        )
                    elif uniform:
                        nc.vector.tensor_scalar_mul(a[:, cs], u[:, cs], glamC[:])
                    else:
                        nc.gpsimd.tensor_mul(a[:, cs], u[:, cs], glamR[:, cs])
                    # q = u * v_next
                    if uniform:
                        nc.gpsimd.tensor_mul(q[:, cs], u[:, cs], vn[:, cs])
                    else:
                        nc.vector.tensor_mul(q[:, cs], u[:, cs], vn[:, cs])
                    # w = q * gamma*(1-lambda) ; b = w + r
                    if uniform:
                        nc.vector.tensor_scalar_mul(q[:, cs], q[:, cs], gmlamC[:])
                    else:
                        nc.vector.tensor_mul(q[:, cs], q[:, cs], gmlamR[:, cs])
                    nc.vector.tensor_add(q[:, cs], q[:, cs], r[:, cs])

                    # backward scan via reversed access patterns: iteration k
                    # reads a/b at time hi-1-k and writes out there too, so
                    # state = a[t]*state + b[t] walks t = hi-1 .. lo.  The
                    # carry enters from v[S] (top chunk) or the previous
                    # chunk's first output column.
                    hi = bounds[pc + 1]
                    if hi == s:
                        init = vn[:, s - 1 : s]
                    else:
                        init = o[:, hi : hi + 1]
                    nc.vector.tensor_tensor_scan(
                        o[:, cs][:, ::-1],
                        a[:, cs][:, ::-1],
                        q[:, cs][:, ::-1],
                        init,
                        op0=ALU.mult,
                        op1=ALU.add,
                    )
                    pending_stores.append((i, ret[rs, cs], o[:, cs]))

            for _, dst, src in pending_stores:
                nc.scalar.dma_start(dst, src)

    nc.compile()
    return nc


_nc_cache = {}


def _get_nc(uniform=True):
    key = ("uni" if uniform else "gen",)
    if key not in _nc_cache:
        _nc_cache[key] = build_kernel(uniform=uniform)
    return _nc_cache[key]


def kernel(values, rewards, dones, raw_gamma, raw_lambd, trace=False):
    values = np.asarray(values, np.float32).reshape(B, S + 1)
    # values[:, 0] is never used by the recurrence: v_next = values[:, 1:],
    # bootstrap carry = values[:, -1]
    vnext = np.ascontiguousarray(values[:, 1:]).astype(np.float16)
    rewards = np.asarray(rewards, np.float32).reshape(B, S).astype(np.float16)
    dones = np.asarray(dones, np.float32).reshape(B, S).astype(FP8_NP)
    g = np.ascontiguousarray(raw_gamma, np.float32).reshape(1, 1)
    lam = np.ascontiguousarray(raw_lambd, np.float32).reshape(1, S)

    # GammaLambdaLearner initializes raw_lambd = atanh(0.9) * ones(S); when
    # every entry is identical the column coefficients collapse to scalars
    # and a cheaper kernel variant applies.  Pure input inspection — the
    # math itself stays on-device in both variants.
    uniform = bool(np.ptp(lam) == 0.0)
    lam_in = lam[:, :1] if uniform else lam

    in_maps = []
    for c in range(N_CORES):
        rs = slice(c * R, (c + 1) * R)
        in_maps.append(
            {
                "vnext": vnext[rs],
                "rewards": rewards[rs],
                "dones": dones[rs],
                "raw_gamma": g,
                "raw_lambd": lam_in,
            }
        )

    nc = _get_nc(uniform)
    if not trace:
        # NTFF profiling needs axon hooks that may be absent; force it off
        # unless explicitly requested
        import os

        os.environ["BASS_NEVER_TRACE"] = "1"
    try:
        res = run_bass_kernel_spmd(
            nc, in_maps, core_ids=list(range(N_CORES)), trace=trace
        )
    except Exception:
        # transient NRT/axon hiccups (e.g. a wedged exec unit from a prior
        # run) are recoverable on retry
        res = run_bass_kernel_spmd(
            nc, in_maps, core_ids=list(range(N_CORES)), trace=trace
        )
    out = np.concatenate([res.results[c]["ret"] for c in range(N_CORES)], axis=0)
    if trace:
        kernel.last_results = res
    return out.astype(np.float32).reshape(B, S, 1)
